# revision 11
# baseline (speedup 1.0000x reference)
"""DifferentialAttention TRN2 kernel v4b: Z fused into attn@v, combine fully
partition-aligned (head 0 stationary [V|1] -> o at partitions 0-63; head 1
stationary [1|V] -> o at partitions 64-127; one SBUF->SBUF DMA shifts each
reciprocal into its o's quadrant, so every DVE op is partition-aligned)."""

import numpy as np

B, T, DM, H, D = 2, 2048, 1024, 16, 64
NCORES = 8
NI = B * T
IB = 512
NIB = T // IB
KO = DM // 128
LAG = 2                   # attn@v lags the scores by this many j-tiles

_cached = {}


def _modules():
    if "mods" in _cached:
        return _cached["mods"]
    import sys
    try:
        import concourse.bass  # noqa: F401
    except ImportError:
        sys.path.insert(0, "/opt/trn_rl_repo")
    import concourse.bacc as bacc
    import concourse.mybir as mybir
    import concourse.tile as tile
    from concourse.bass_utils import run_bass_kernel_spmd
    _cached["mods"] = (bacc, mybir, tile, run_bass_kernel_spmd)
    return _cached["mods"]


def build_nc(reps=1):
    key = ("nc", reps)
    if key in _cached:
        return _cached[key]
    bacc, mybir, tile, _ = _modules()
    f32 = mybir.dt.float32
    f32r = mybir.dt.float32r
    bf16 = mybir.dt.bfloat16
    EXP = mybir.ActivationFunctionType.Exp
    MUL = mybir.AluOpType.mult
    SUB = mybir.AluOpType.subtract

    nc = bacc.Bacc(None, target_bir_lowering=False, debug=False)

    xt = nc.dram_tensor("xt", [DM, NI], f32, kind="ExternalInput")
    wq = nc.dram_tensor("wq", [DM, 5 * 128], f32, kind="ExternalInput")
    wo = nc.dram_tensor("wo", [128, DM], f32, kind="ExternalInput")
    tri8 = nc.dram_tensor("tri8", [128, 128], f32, kind="ExternalInput")
    lam2 = nc.dram_tensor("lam2", [128, 2], f32, kind="ExternalInput")
    y = nc.dram_tensor("y", [NI, DM], f32, kind="ExternalOutput")

    xt_r = xt.rearrange("(ko ki) n -> ki ko n", ki=128)
    wq_r = wq.rearrange("(ko ki) c -> ki ko c", ki=128)

    with tile.TileContext(nc) as tc:
        with (
            tc.tile_pool(name="const", bufs=1) as const,
            tc.tile_pool(name="xts", bufs=2) as xts_p,
            tc.tile_pool(name="pp", bufs=2 + LAG) as pp_p,
            tc.tile_pool(name="rb", bufs=4) as rb_p,
            tc.tile_pool(name="rs", bufs=4) as rs_p,
            tc.tile_pool(name="of", bufs=2) as of_p,
            tc.tile_pool(name="tmp", bufs=4) as tmp_p,
            tc.tile_pool(name="ys", bufs=3) as ys_p,
            tc.tile_pool(name="psw", bufs=2, space="PSUM") as psw,
            tc.tile_pool(name="psz", bufs=2, space="PSUM") as psz,
            tc.tile_pool(name="pso", bufs=1, space="PSUM") as pso_p,
            tc.tile_pool(name="psq", bufs=1, space="PSUM") as psq,
        ):
            WQ = const.tile([128, KO, 5 * 128], bf16)
            nc.gpsimd.dma_start(WQ[:], wq_r)
            WO = const.tile([128, DM], f32r)
            nc.sync.dma_start(WO[:], wo[:].bitcast(f32r))
            TRI8 = const.tile([128, 128], bf16)
            nc.gpsimd.dma_start(TRI8[:], tri8[:])
            LAM2 = const.tile([128, 2], f32)
            nc.sync.dma_start(LAM2[:], lam2[:])
            QS = const.tile([128, 2, NI], bf16)
            KPAD = [[const.tile([128, NI], bf16, name=f"kp{p}{h}")
                     for h in range(2)] for p in range(2)]
            for p in range(2):
                for h in range(2):
                    nc.gpsimd.memset(KPAD[p][h][:], 0.0)
            # VN2[j, jtile, 0, :] = [v_h0 | 1];  VN2[j, jtile, 1, :] = [1 | v_h1]
            VN2 = const.tile([128, B * 16, 2, 128], bf16)
            nc.gpsimd.memset(VN2[:, :, 0, 64:128], 1.0)
            nc.gpsimd.memset(VN2[:, :, 1, 0:64], 1.0)

            def make_qkv_emitters(b, ib):
                i0 = b * T + ib * IB
                state = {}

                def dma():
                    xts = xts_p.tile([128, KO, IB], bf16)
                    nc.gpsimd.dma_start(xts[:], xt_r[:, :, i0:i0 + IB])
                    state["xts"] = xts

                ems = [(False, dma)]

                def mm(c, k):
                    if k == 0:
                        state["ps"] = psq.tile([128, IB], f32, tag="q",
                                               name="qkv_ps")
                    nc.tensor.matmul(
                        state["ps"][:, 0:IB],
                        WQ[:, k, c * 128:(c + 1) * 128],
                        state["xts"][:, k, :],
                        start=(k == 0),
                        stop=(k == KO - 1),
                    )

                def out_q(c):
                    nc.vector.tensor_copy(QS[:, c, i0:i0 + IB],
                                          state["ps"][:, 0:IB])

                def out_k(c):
                    for h in range(2):
                        nc.vector.tensor_copy(
                            KPAD[c - 2][h][h * 64:(h + 1) * 64, i0:i0 + IB],
                            state["ps"][h * 64:(h + 1) * 64, 0:IB],
                        )

                def mm_v(tb, k):
                    if k == 0:
                        state["ps"] = psq.tile([128, IB], f32, tag="q",
                                               name="qkv_ps")
                    nc.tensor.matmul(
                        state["ps"][:, 0:128],
                        state["xts"][:, k, tb * 128:(tb + 1) * 128],
                        WQ[:, k, 4 * 128:5 * 128],
                        start=(k == 0),
                        stop=(k == KO - 1),
                    )

                def out_v(tb):
                    t = b * 16 + ib * 4 + tb
                    nc.vector.tensor_copy(
                        VN2[:, t, 0, 0:64], state["ps"][:, 0:64])
                    nc.vector.tensor_copy(
                        VN2[:, t, 1, 64:128], state["ps"][:, 64:128])

                for c in range(4):
                    for k in range(KO):
                        ems.append((True, lambda c=c, k=k: mm(c, k)))
                    if c < 2:
                        ems.append((False, lambda c=c: out_q(c)))
                    else:
                        ems.append((False, lambda c=c: out_k(c)))
                for tb in range(IB // 128):
                    for k in range(KO):
                        ems.append((True, lambda tb=tb, k=k: mm_v(tb, k)))
                    ems.append((False, lambda tb=tb: out_v(tb)))
                return ems

            def make_outproj_emitters(i0, OF):
                ems = []

                def step(ic):
                    ysb = ys_p.tile([128, 1024], f32)
                    for nt in range(DM // 512):
                        pso = pso_p.tile([128, IB], f32, tag="o",
                                         name=f"pso{nt}")
                        nc.tensor.matmul(
                            pso[:],
                            OF[:, ic * 128:(ic + 1) * 128],
                            WO[:, nt * 512:(nt + 1) * 512],
                            start=True,
                            stop=True,
                        )
                        nc.vector.tensor_copy(
                            ysb[:, nt * 512:(nt + 1) * 512], pso[:]
                        )
                    nc.sync.dma_start(
                        y[i0 + ic * 128:i0 + (ic + 1) * 128, :], ysb[:]
                    )

                for ic in range(IB // 128):
                    ems.append((True, lambda ic=ic: step(ic)))
                return ems

            pairs = [(b, ib) for b in range(B) for ib in range(NIB)] * reps
            for _, em in make_qkv_emitters(*pairs[0]):
                em()
            pend_op = []
            for idx, (b, ib) in enumerate(pairs):
                    i0 = b * T + ib * IB
                    pend = list(pend_op)
                    pend_op = []
                    if idx + 1 < len(pairs):
                        pend += make_qkv_emitters(*pairs[idx + 1])
                    njt = 4 * ib + 4

                    def inject():
                        while pend:
                            is_pe, em = pend.pop(0)
                            em()
                            if is_pe:
                                break

                    OF = of_p.tile([128, IB], f32r)
                    for h in range(2):
                        # o rows for this head: 0-63 (h0) / 64-127 (h1);
                        # Z-replica rows are the other quadrant pair
                        osl = slice(64 * h, 64 * h + 64)
                        zsl = slice(64 - 64 * h, 128 - 64 * h)
                        oz = [psz.tile([128, IB], f32, tag="acc",
                                       name=f"oz{p}{h}") for p in range(2)]
                        avq = []

                        def av(jt, pp_t, h=h, oz=oz, b=b, ib=ib, njt=njt):
                            r = jt - 4 * ib
                            lo = 128 * r if r > 0 else 0
                            for p in range(2):
                                nc.tensor.matmul(
                                    oz[p][:, lo:IB],
                                    VN2[:, b * 16 + jt, h, :],
                                    pp_t[:, p, lo:IB],
                                    start=(jt == 0),
                                    stop=(jt == njt - 1),
                                    skip_group_check=True,
                                )

                        for jt in range(njt):
                            r = jt - 4 * ib
                            lo = 128 * r if r > 0 else 0
                            jq = b * T + jt * 128
                            st = psw.tile([128, 2, IB], f32, tag="w")
                            for p in range(2):
                                nc.tensor.matmul(
                                    st[:, p, lo:IB],
                                    KPAD[p][h][:, jq:jq + 128],
                                    QS[:, p, i0 + lo:i0 + IB],
                                    start=True,
                                    stop=True,
                                )
                            pp = pp_p.tile([128, 2, IB], bf16, tag="pp")
                            nc.scalar.activation(
                                pp[:, :, lo:IB], st[:, :, lo:IB], EXP,
                                scale=0.125,
                            )
                            if r >= 0:
                                nc.vector.tensor_tensor(
                                    pp[:, :, lo:lo + 128],
                                    pp[:, :, lo:lo + 128],
                                    TRI8[:, None, :].to_broadcast(
                                        (128, 2, 128)),
                                    MUL,
                                )
                            avq.append((jt, pp))
                            if len(avq) > LAG:
                                av(*avq.pop(0))
                            inject()
                        while avq:
                            av(*avq.pop(0))
                            inject()

                        # normalizers: reciprocal in the Z quadrant (aligned),
                        # then one DMA shifts it into the o quadrant
                        rs = [None, None]
                        for p in range(2):
                            r_t = rb_p.tile([128, IB], f32, tag="rb",
                                            name=f"rb{p}{h}")
                            nc.vector.reciprocal(r_t[zsl, :], oz[p][zsl, :])
                            if p == 1:
                                nc.vector.tensor_scalar(
                                    r_t[zsl, :], r_t[zsl, :],
                                    LAM2[zsl, h:h + 1], None, MUL,
                                )
                            s_t = rs_p.tile([128, IB], f32, tag="rs",
                                            name=f"rs{p}{h}")
                            nc.sync.dma_start(s_t[osl, :], r_t[zsl, :])
                            rs[p] = s_t
                        t1 = tmp_p.tile([128, IB], f32, tag="t")
                        t2 = tmp_p.tile([128, IB], f32, tag="t")
                        nc.vector.tensor_tensor(
                            t1[osl, :], oz[0][osl, :], rs[0][osl, :], MUL)
                        nc.vector.tensor_tensor(
                            t2[osl, :], oz[1][osl, :], rs[1][osl, :], MUL)
                        nc.vector.tensor_tensor(
                            OF[osl, :], t1[osl, :], t2[osl, :], SUB)
                    while pend:
                        pend.pop(0)[1]()

                    op_ems = make_outproj_emitters(i0, OF)
                    if idx + 1 < len(pairs):
                        pend_op = op_ems
                    else:
                        for _, em in op_ems:
                            em()

    nc.compile()
    _cached[key] = nc
    return nc


def make_in_maps(x, mask, W_qkv, W_out, lam):
    x = np.asarray(x, dtype=np.float32)
    mask = np.asarray(mask, dtype=np.float32)
    W_qkv = np.asarray(W_qkv, dtype=np.float32)
    W_out = np.asarray(W_out, dtype=np.float32)
    lam = np.asarray(lam, dtype=np.float32)

    xt = np.ascontiguousarray(x.reshape(NI, DM).T)
    tri8 = np.ascontiguousarray(
        (mask[0, 0, :128, :128].T == 0.0).astype(np.float32)
    )
    lam_c = np.clip(lam, 0.0, 1.0)
    Wr = W_qkv.reshape(DM, H, 5, D)
    Wo_r = W_out.reshape(H, D, DM)
    in_maps = []
    for c in range(NCORES):
        hA, hB = 2 * c, 2 * c + 1
        wq_loc = np.ascontiguousarray(
            np.concatenate(
                [Wr[:, [hA, hB], t, :].reshape(DM, 2 * D) for t in range(5)],
                axis=1,
            )
        )
        wo_loc = np.ascontiguousarray(Wo_r[[hA, hB]].reshape(2 * D, DM))
        lam2 = np.empty((128, 2), dtype=np.float32)
        lam2[:, 0] = lam_c[hA]
        lam2[:, 1] = lam_c[hB]
        in_maps.append(
            {"xt": xt, "wq": wq_loc, "wo": wo_loc, "tri8": tri8,
             "lam2": lam2}
        )
    return in_maps


def kernel(x, mask, W_qkv, W_out, lam):
    _, _, _, run_bass_kernel_spmd = _modules()
    nc = build_nc()
    in_maps = make_in_maps(x, mask, W_qkv, W_out, lam)
    res = run_bass_kernel_spmd(nc, in_maps, core_ids=list(range(NCORES)))
    parts = [res.results[c]["y"] for c in range(NCORES)]
    yy = parts[0].astype(np.float64)
    for p in parts[1:]:
        yy = yy + p
    return yy.astype(np.float32).reshape(B, T, DM)


# revision 12
# speedup vs baseline: 1.0196x; 1.0196x over previous
"""DifferentialAttention TRN2 kernel v4b: Z fused into attn@v, combine fully
partition-aligned (head 0 stationary [V|1] -> o at partitions 0-63; head 1
stationary [1|V] -> o at partitions 64-127; one SBUF->SBUF DMA shifts each
reciprocal into its o's quadrant, so every DVE op is partition-aligned)."""

import numpy as np

B, T, DM, H, D = 2, 2048, 1024, 16, 64
NCORES = 8
NI = B * T
IB = 512
NIB = T // IB
KO = DM // 128
LAG = 3                   # attn@v lags the scores by this many j-tiles

_cached = {}


def _modules():
    if "mods" in _cached:
        return _cached["mods"]
    import sys
    try:
        import concourse.bass  # noqa: F401
    except ImportError:
        sys.path.insert(0, "/opt/trn_rl_repo")
    import concourse.bacc as bacc
    import concourse.mybir as mybir
    import concourse.tile as tile
    from concourse.bass_utils import run_bass_kernel_spmd
    _cached["mods"] = (bacc, mybir, tile, run_bass_kernel_spmd)
    return _cached["mods"]


def build_nc(reps=1):
    key = ("nc", reps)
    if key in _cached:
        return _cached[key]
    bacc, mybir, tile, _ = _modules()
    f32 = mybir.dt.float32
    f32r = mybir.dt.float32r
    bf16 = mybir.dt.bfloat16
    EXP = mybir.ActivationFunctionType.Exp
    MUL = mybir.AluOpType.mult
    SUB = mybir.AluOpType.subtract

    nc = bacc.Bacc(None, target_bir_lowering=False, debug=False)

    xt = nc.dram_tensor("xt", [DM, NI], f32, kind="ExternalInput")
    wq = nc.dram_tensor("wq", [DM, 5 * 128], f32, kind="ExternalInput")
    wo = nc.dram_tensor("wo", [128, DM], f32, kind="ExternalInput")
    tri8 = nc.dram_tensor("tri8", [128, 128], f32, kind="ExternalInput")
    lam2 = nc.dram_tensor("lam2", [128, 2], f32, kind="ExternalInput")
    y = nc.dram_tensor("y", [NI, DM], f32, kind="ExternalOutput")

    xt_r = xt.rearrange("(ko ki) n -> ki ko n", ki=128)
    wq_r = wq.rearrange("(ko ki) c -> ki ko c", ki=128)

    with tile.TileContext(nc) as tc:
        with (
            tc.tile_pool(name="const", bufs=1) as const,
            tc.tile_pool(name="xts", bufs=2) as xts_p,
            tc.tile_pool(name="pp", bufs=2 + LAG) as pp_p,
            tc.tile_pool(name="rb", bufs=4) as rb_p,
            tc.tile_pool(name="rs", bufs=4) as rs_p,
            tc.tile_pool(name="of", bufs=2) as of_p,
            tc.tile_pool(name="tmp", bufs=4) as tmp_p,
            tc.tile_pool(name="ys", bufs=3) as ys_p,
            tc.tile_pool(name="psw", bufs=2, space="PSUM") as psw,
            tc.tile_pool(name="psz", bufs=2, space="PSUM") as psz,
            tc.tile_pool(name="pso", bufs=1, space="PSUM") as pso_p,
            tc.tile_pool(name="psq", bufs=1, space="PSUM") as psq,
        ):
            WQ = const.tile([128, KO, 5 * 128], bf16)
            nc.gpsimd.dma_start(WQ[:], wq_r)
            WO = const.tile([128, DM], f32r)
            nc.sync.dma_start(WO[:], wo[:].bitcast(f32r))
            TRI8 = const.tile([128, 128], bf16)
            nc.gpsimd.dma_start(TRI8[:], tri8[:])
            LAM2 = const.tile([128, 2], f32)
            nc.sync.dma_start(LAM2[:], lam2[:])
            QS = const.tile([128, 2, NI], bf16)
            KPAD = [[const.tile([128, NI], bf16, name=f"kp{p}{h}")
                     for h in range(2)] for p in range(2)]
            for p in range(2):
                for h in range(2):
                    nc.gpsimd.memset(KPAD[p][h][:], 0.0)
            # VN2[j, jtile, 0, :] = [v_h0 | 1];  VN2[j, jtile, 1, :] = [1 | v_h1]
            VN2 = const.tile([128, B * 16, 2, 128], bf16)
            nc.gpsimd.memset(VN2[:, :, 0, 64:128], 1.0)
            nc.gpsimd.memset(VN2[:, :, 1, 0:64], 1.0)

            def make_qkv_emitters(b, ib):
                i0 = b * T + ib * IB
                state = {}

                def dma():
                    xts = xts_p.tile([128, KO, IB], bf16)
                    nc.gpsimd.dma_start(xts[:], xt_r[:, :, i0:i0 + IB])
                    state["xts"] = xts

                ems = [(False, dma)]

                def mm(c, k):
                    if k == 0:
                        state["ps"] = psq.tile([128, IB], f32, tag="q",
                                               name="qkv_ps")
                    nc.tensor.matmul(
                        state["ps"][:, 0:IB],
                        WQ[:, k, c * 128:(c + 1) * 128],
                        state["xts"][:, k, :],
                        start=(k == 0),
                        stop=(k == KO - 1),
                    )

                def out_q(c):
                    nc.vector.tensor_copy(QS[:, c, i0:i0 + IB],
                                          state["ps"][:, 0:IB])

                def out_k(c):
                    for h in range(2):
                        nc.vector.tensor_copy(
                            KPAD[c - 2][h][h * 64:(h + 1) * 64, i0:i0 + IB],
                            state["ps"][h * 64:(h + 1) * 64, 0:IB],
                        )

                def mm_v(tb, k):
                    if k == 0:
                        state["ps"] = psq.tile([128, IB], f32, tag="q",
                                               name="qkv_ps")
                    nc.tensor.matmul(
                        state["ps"][:, 0:128],
                        state["xts"][:, k, tb * 128:(tb + 1) * 128],
                        WQ[:, k, 4 * 128:5 * 128],
                        start=(k == 0),
                        stop=(k == KO - 1),
                    )

                def out_v(tb):
                    t = b * 16 + ib * 4 + tb
                    nc.vector.tensor_copy(
                        VN2[:, t, 0, 0:64], state["ps"][:, 0:64])
                    nc.vector.tensor_copy(
                        VN2[:, t, 1, 64:128], state["ps"][:, 64:128])

                for c in range(4):
                    for k in range(KO):
                        ems.append((True, lambda c=c, k=k: mm(c, k)))
                    if c < 2:
                        ems.append((False, lambda c=c: out_q(c)))
                    else:
                        ems.append((False, lambda c=c: out_k(c)))
                for tb in range(IB // 128):
                    for k in range(KO):
                        ems.append((True, lambda tb=tb, k=k: mm_v(tb, k)))
                    ems.append((False, lambda tb=tb: out_v(tb)))
                return ems

            def make_outproj_emitters(i0, OF):
                ems = []

                def step(ic):
                    ysb = ys_p.tile([128, 1024], f32)
                    for nt in range(DM // 512):
                        pso = pso_p.tile([128, IB], f32, tag="o",
                                         name=f"pso{nt}")
                        nc.tensor.matmul(
                            pso[:],
                            OF[:, ic * 128:(ic + 1) * 128],
                            WO[:, nt * 512:(nt + 1) * 512],
                            start=True,
                            stop=True,
                        )
                        nc.vector.tensor_copy(
                            ysb[:, nt * 512:(nt + 1) * 512], pso[:]
                        )
                    nc.sync.dma_start(
                        y[i0 + ic * 128:i0 + (ic + 1) * 128, :], ysb[:]
                    )

                for ic in range(IB // 128):
                    ems.append((True, lambda ic=ic: step(ic)))
                return ems

            pairs = [(b, ib) for b in range(B) for ib in range(NIB)] * reps
            for _, em in make_qkv_emitters(*pairs[0]):
                em()
            pend_op = []
            for idx, (b, ib) in enumerate(pairs):
                    i0 = b * T + ib * IB
                    pend = list(pend_op)
                    pend_op = []
                    if idx + 1 < len(pairs):
                        pend += make_qkv_emitters(*pairs[idx + 1])
                    njt = 4 * ib + 4

                    def inject():
                        while pend:
                            is_pe, em = pend.pop(0)
                            em()
                            if is_pe:
                                break

                    OF = of_p.tile([128, IB], f32r)
                    for h in range(2):
                        # o rows for this head: 0-63 (h0) / 64-127 (h1);
                        # Z-replica rows are the other quadrant pair
                        osl = slice(64 * h, 64 * h + 64)
                        zsl = slice(64 - 64 * h, 128 - 64 * h)
                        oz = [psz.tile([128, IB], f32, tag="acc",
                                       name=f"oz{p}{h}") for p in range(2)]
                        avq = []

                        def av(jt, pp_t, h=h, oz=oz, b=b, ib=ib, njt=njt):
                            r = jt - 4 * ib
                            lo = 128 * r if r > 0 else 0
                            for p in range(2):
                                nc.tensor.matmul(
                                    oz[p][:, lo:IB],
                                    VN2[:, b * 16 + jt, h, :],
                                    pp_t[:, p, lo:IB],
                                    start=(jt == 0),
                                    stop=(jt == njt - 1),
                                    skip_group_check=True,
                                )

                        for jt in range(njt):
                            r = jt - 4 * ib
                            lo = 128 * r if r > 0 else 0
                            jq = b * T + jt * 128
                            st = psw.tile([128, 2, IB], f32, tag="w")
                            for p in range(2):
                                nc.tensor.matmul(
                                    st[:, p, lo:IB],
                                    KPAD[p][h][:, jq:jq + 128],
                                    QS[:, p, i0 + lo:i0 + IB],
                                    start=True,
                                    stop=True,
                                )
                            pp = pp_p.tile([128, 2, IB], bf16, tag="pp")
                            nc.scalar.activation(
                                pp[:, :, lo:IB], st[:, :, lo:IB], EXP,
                                scale=0.125,
                            )
                            if r >= 0:
                                nc.gpsimd.tensor_tensor(
                                    pp[:, :, lo:lo + 128],
                                    pp[:, :, lo:lo + 128],
                                    TRI8[:, None, :].to_broadcast(
                                        (128, 2, 128)),
                                    MUL,
                                )
                            avq.append((jt, pp))
                            if len(avq) > LAG:
                                av(*avq.pop(0))
                            inject()
                        while avq:
                            av(*avq.pop(0))

                        # normalizers: reciprocal in the Z quadrant (aligned),
                        # then one DMA shifts it into the o quadrant
                        rs = [None, None]
                        for p in range(2):
                            r_t = rb_p.tile([128, IB], f32, tag="rb",
                                            name=f"rb{p}{h}")
                            nc.vector.reciprocal(r_t[zsl, :], oz[p][zsl, :])
                            if p == 1:
                                nc.vector.tensor_scalar(
                                    r_t[zsl, :], r_t[zsl, :],
                                    LAM2[zsl, h:h + 1], None, MUL,
                                )
                            s_t = rs_p.tile([128, IB], f32, tag="rs",
                                            name=f"rs{p}{h}")
                            nc.sync.dma_start(s_t[osl, :], r_t[zsl, :])
                            rs[p] = s_t
                        t1 = tmp_p.tile([128, IB], f32, tag="t")
                        t2 = tmp_p.tile([128, IB], f32, tag="t")
                        nc.vector.tensor_tensor(
                            t1[osl, :], oz[0][osl, :], rs[0][osl, :], MUL)
                        nc.vector.tensor_tensor(
                            t2[osl, :], oz[1][osl, :], rs[1][osl, :], MUL)
                        nc.vector.tensor_tensor(
                            OF[osl, :], t1[osl, :], t2[osl, :], SUB)
                    while pend:
                        pend.pop(0)[1]()

                    op_ems = make_outproj_emitters(i0, OF)
                    if idx + 1 < len(pairs):
                        pend_op = op_ems
                    else:
                        for _, em in op_ems:
                            em()

    nc.compile()
    _cached[key] = nc
    return nc


def make_in_maps(x, mask, W_qkv, W_out, lam):
    x = np.asarray(x, dtype=np.float32)
    mask = np.asarray(mask, dtype=np.float32)
    W_qkv = np.asarray(W_qkv, dtype=np.float32)
    W_out = np.asarray(W_out, dtype=np.float32)
    lam = np.asarray(lam, dtype=np.float32)

    xt = np.ascontiguousarray(x.reshape(NI, DM).T)
    tri8 = np.ascontiguousarray(
        (mask[0, 0, :128, :128].T == 0.0).astype(np.float32)
    )
    lam_c = np.clip(lam, 0.0, 1.0)
    Wr = W_qkv.reshape(DM, H, 5, D)
    Wo_r = W_out.reshape(H, D, DM)
    in_maps = []
    for c in range(NCORES):
        hA, hB = 2 * c, 2 * c + 1
        wq_loc = np.ascontiguousarray(
            np.concatenate(
                [Wr[:, [hA, hB], t, :].reshape(DM, 2 * D) for t in range(5)],
                axis=1,
            )
        )
        wo_loc = np.ascontiguousarray(Wo_r[[hA, hB]].reshape(2 * D, DM))
        lam2 = np.empty((128, 2), dtype=np.float32)
        lam2[:, 0] = lam_c[hA]
        lam2[:, 1] = lam_c[hB]
        in_maps.append(
            {"xt": xt, "wq": wq_loc, "wo": wo_loc, "tri8": tri8,
             "lam2": lam2}
        )
    return in_maps


def kernel(x, mask, W_qkv, W_out, lam):
    _, _, _, run_bass_kernel_spmd = _modules()
    nc = build_nc()
    in_maps = make_in_maps(x, mask, W_qkv, W_out, lam)
    res = run_bass_kernel_spmd(nc, in_maps, core_ids=list(range(NCORES)))
    parts = [res.results[c]["y"] for c in range(NCORES)]
    yy = parts[0].astype(np.float64)
    for p in parts[1:]:
        yy = yy + p
    return yy.astype(np.float32).reshape(B, T, DM)
